# revision 7
# baseline (speedup 1.0000x reference)
"""GQA attention layer (B=2, S=2048, D=4096, 32 Q heads / 8 KV heads, HD=128)
with rotary embeddings, causal mask, and output projection, on 8 trn2 cores.

Sharding: tensor-parallel over heads for QKV+attention (core c owns Q heads
[4c,4c+4) and KV head c), two AllToAlls (split by head-pair) re-shard the
attention output from head-sharded to token-sharded, then a token-sharded
output projection with the full wo. Host gathers the 8 token shards.

Schedule highlights (single TileContext, manual emission interleaving):
 - bf16 datapath end to end (fp32 PSUM accumulate); rel err ~8e-3.
 - qkvT m-tiles and v stay SBUF-resident; no DRAM round trip; v is
   transposed to [tok, hd] blocks on the DMA xbar (dma_start_transpose).
 - RoPE pair-swap as an SBUF->SBUF partition-swap DMA with the rotation
   signs folded into sinP host-side; cos/sin multiplies on DVE.
 - Softmax is computed in [k, q] layout; denominators via a bf16 pairwise
   tree-add on DVE + one Pool partition_all_reduce per q-tile (no PE
   ones-matmuls); fast approximate reciprocal on DVE.
 - Attention units (head, batch, q-tile) are emitted as generators and
   woven between QKV matmul octets with a PE-time-weighted filler so the
   PE never drains behind ACT exp latency; y-matmuls lag two kb-pairs.
 - nt0 runs quad-major across six concurrent PSUM accumulators (borrowing
   idle psc banks) so startup follows DMA arrival.
 - Phase 4 runs two passes: hp0 head-blocks (needs only A2A#1) serve as PE
   filler under the ACT-bound attention tail, spilling bf16 partials to
   DRAM; after A2A#2 the hp1 blocks are added to the reloaded partials.
"""
import sys

sys.path.insert(0, "/opt/trn_rl_repo")

import numpy as np

B, S, D = 2, 2048, 4096
NH, NL, HD = 32, 8, 128
CORES = 8
QH = NH // CORES          # 4 q heads per core
TOK = B * S               # 4096
TPC = TOK // CORES        # 512 tokens per core (output sharding)
NT = 512                  # phase-1 token block width
NNT = TOK // NT           # 8 token blocks (b0: 0..3, b1: 4..7)
QT_W = 512                # attention q tile width
N_QT = S // QT_W          # 4 q tiles per batch
N_KB = S // 128           # 16 k blocks per batch
WO_NT = 512               # phase-4 dout block width
KB_D = D // 128           # 32 contraction blocks over D
NPAIR = KB_D // 2         # 16 kb pairs for weight packing
NROW = (QH + 2) * HD      # 768 qkv rows per core
NM = NROW // 128          # 6 m tiles (0..3 q heads, 4 kT, 5 vT)
SCALE = 1.0 / np.sqrt(np.float32(HD))

_CACHE = {}


def _build_nc(mode, c_sub, sim=False):
    """mode: 'causal' | 'full' | 'generic'. c_sub: global softmax shift.
    sim=True: single-core TimelineSim variant (collective replaced by DMAs)."""
    import concourse.bacc as bacc
    import concourse.mybir as mybir
    import concourse.tile as tile
    import concourse.bass_isa as bass_isa
    import ml_dtypes
    from contextlib import ExitStack

    F32 = mybir.dt.float32
    BF16 = mybir.dt.bfloat16
    AT = mybir.ActivationFunctionType
    OP = mybir.AluOpType

    nc = bacc.Bacc("TRN2", target_bir_lowering=False, debug=False,
                   num_devices=1 if sim else CORES)

    xT_d = nc.dram_tensor("xT", (D, TOK), BF16, kind="ExternalInput").ap()
    wqkvT_d = nc.dram_tensor("wqkvT", (D, NROW), BF16,
                             kind="ExternalInput").ap()
    woT_d = nc.dram_tensor("woT", (D, D), BF16, kind="ExternalInput").ap()
    # cos/sin stacked: csP[:, 0, :] = cos (hd-pair expanded), [:, 1, :] =
    # sign-folded sin (see host prep)
    csP_d = nc.dram_tensor("csP", (128, 2, TOK), BF16, kind="ExternalInput").ap()
    if mode == "generic":
        biasT_d = nc.dram_tensor("biasT", (S, S), BF16, kind="ExternalInput").ap()
    out_d = nc.dram_tensor("out", (TPC, D), BF16, kind="ExternalOutput").ap()

    # within-diag-block causal mask: tri[p, c] = 1 iff c >= p
    tri = np.triu(np.ones((128, 128), dtype=np.float32))
    tri_h = nc.inline_tensor(tri.astype(ml_dtypes.bfloat16), name="trimask")

    with tile.TileContext(nc) as tc, ExitStack() as glob:
        dram = glob.enter_context(tc.tile_pool(name="dram", bufs=1, space="DRAM"))
        consts = glob.enter_context(tc.tile_pool(name="consts", bufs=1))
        persist = glob.enter_context(tc.tile_pool(name="persist", bufs=1))

        # split A2A: hp=0 carries head-locals {0,1}, hp=1 carries {2,3}
        a2a_in = [dram.tile([TOK // 2, TPC], BF16, name=f"a2a_in{hp}")
                  for hp in range(2)]
        part_d = dram.tile([TPC, D], BF16, name="part_d")
        a2a_out = [dram.tile([TOK // 2, TPC], BF16, name=f"a2a_out{hp}")
                   for hp in range(2)]

        tri_sb = consts.tile([128, 128], BF16)

        if c_sub != 0.0:
            _bias_t = consts.tile([128, 1], mybir.dt.float32, name="expb")
            nc.gpsimd.memset(_bias_t[:], -float(c_sub))
            exp_bias = _bias_t[:]
        else:
            exp_bias = 0.0

        def emit_const_dmas():
            nc.sync.dma_start(tri_sb[:], tri_h.ap())

        # persistent SBUF: roped qkvT m-tiles (m 0..3 q heads, 4 = kT) and
        # v in natural [tok, hd] block layout
        qkv_sb = [persist.tile([128, TOK], BF16, name=f"qkv{m}")
                  for m in range(5)]
        v_sb = [persist.tile([128, NT // 128, HD], BF16, name=f"v{nt}")
                for nt in range(NNT)]

        def v_blk(b, kb):
            t0 = S * b + 128 * kb
            return v_sb[t0 // NT][:, (t0 % NT) // 128, :]

        # ---------------- attention pools (live until tail units done)
        p2_at = glob.enter_context(tc.tile_pool(name="p2_at", bufs=4))
        p2_tr = glob.enter_context(tc.tile_pool(name="p2_tr", bufs=6))
        p2_nm = glob.enter_context(tc.tile_pool(name="p2_nm", bufs=2))
        p2_y = glob.enter_context(tc.tile_pool(name="p2_y", bufs=2))
        p2_psc = glob.enter_context(tc.tile_pool(name="p2_psc", bufs=2, space="PSUM"))
        p2_py = glob.enter_context(tc.tile_pool(name="p2_py", bufs=2, space="PSUM"))
        if mode == "generic":
            p2_bias = glob.enter_context(tc.tile_pool(name="p2_bias", bufs=2))

        # ---------------- attention unit emission (generator of steps)
        def attn_unit(h, b, qt):
            """Emit one (head, batch, q-tile) attention unit; yields between
            kb-pair steps so filler matmuls can be woven in."""
            hp = h // 2
            t0 = S * b
            q0 = t0 + QT_W * qt
            kb_max = 4 * qt + 4 if mode == "causal" else N_KB
            n_pair = kb_max // 2
            kT = qkv_sb[4]
            qT = qkv_sb[h]

            py = p2_py.tile([128, QT_W], F32, name="py")
            stack = []          # binary-counter tree accumulator [(level, tile)]
            pend = []           # deferred y-matmul closures (run 1 step late)

            def tree_add(a, bt):
                s = p2_tr.tile([128, QT_W], BF16, name="ts")
                with nc.allow_low_precision(reason="bf16 denom tree"):
                    nc.vector.tensor_tensor(s[:], a[:], bt[:], op=OP.add)
                return s

            def tree_push(t, level):
                stack.append((level, t))
                while len(stack) >= 2 and stack[-1][0] == stack[-2][0]:
                    l1, a = stack.pop()
                    _, bt = stack.pop()
                    tree_push_flat(l1 + 1, tree_add(a, bt))

            def tree_push_flat(level, t):
                stack.append((level, t))
                while len(stack) >= 2 and stack[-1][0] == stack[-2][0]:
                    l1, a = stack.pop()
                    _, bt = stack.pop()
                    stack.append((l1 + 1, tree_add(a, bt)))

            for pr in range(n_pair):
                kb0, kb1 = 2 * pr, 2 * pr + 1
                psc = p2_psc.tile([128, 2 * QT_W], F32, name="psc")
                at = p2_at.tile([128, 2 * QT_W], BF16, name="at")
                # diag narrowing: kb is in the diag region iff kb >= 4*qt
                c00 = 128 * (kb0 - 4 * qt) if (
                    mode == "causal" and kb0 >= 4 * qt) else 0
                c01 = 128 * (kb1 - 4 * qt) if (
                    mode == "causal" and kb1 >= 4 * qt) else 0
                nc.tensor.matmul(psc[:, c00:QT_W],
                                 kT[:, t0 + 128 * kb0:t0 + 128 * (kb0 + 1)],
                                 qT[:, q0 + c00:q0 + QT_W],
                                 start=True, stop=True)
                nc.tensor.matmul(psc[:, QT_W + c01:2 * QT_W],
                                 kT[:, t0 + 128 * kb1:t0 + 128 * (kb1 + 1)],
                                 qT[:, q0 + c01:q0 + QT_W],
                                 start=True, stop=True, skip_group_check=True)
                if mode == "generic":
                    bt_t = p2_bias.tile([128, 2, QT_W], BF16, name="bt")
                    nc.sync.dma_start(
                        bt_t[:], biasT_d[128 * kb0:128 * (kb0 + 2),
                                         QT_W * qt:QT_W * (qt + 1)]
                        .rearrange("(two p) c -> p two c", p=128))
                    nc.vector.tensor_tensor(
                        psc[:].rearrange("p (two c) -> p two c", two=2),
                        psc[:].rearrange("p (two c) -> p two c", two=2),
                        bt_t[:], op=OP.add)
                with nc.allow_low_precision(reason="bf16 attn weights"):
                    if c00 == 0 and c01 == 0:
                        nc.scalar.activation(at[:], psc[:], AT.Exp,
                                             bias=exp_bias,
                                             scale=float(SCALE))
                    else:
                        nc.scalar.activation(at[:, c00:QT_W], psc[:, c00:QT_W],
                                             AT.Exp, bias=exp_bias,
                                             scale=float(SCALE))
                        nc.scalar.activation(
                            at[:, QT_W + c01:2 * QT_W],
                            psc[:, QT_W + c01:2 * QT_W], AT.Exp,
                            bias=exp_bias, scale=float(SCALE))
                        # zero the unwritten holes so tree-adds stay finite
                        if c00 > 0:
                            nc.gpsimd.memset(at[:, 0:c00], 0.0)
                        if c01 > 0:
                            nc.gpsimd.memset(at[:, QT_W:QT_W + c01], 0.0)
                    # within-diag-block causal mask
                    if mode == "causal" and kb0 >= 4 * qt:
                        nc.gpsimd.tensor_tensor(
                            at[:, c00:c00 + 128], at[:, c00:c00 + 128],
                            tri_sb[:], op=OP.mult)
                    if mode == "causal" and kb1 >= 4 * qt:
                        nc.gpsimd.tensor_tensor(
                            at[:, QT_W + c01:QT_W + c01 + 128],
                            at[:, QT_W + c01:QT_W + c01 + 128],
                            tri_sb[:], op=OP.mult)

                def y_mms(kb0=kb0, kb1=kb1, at=at, c00=c00, c01=c01):
                    nc.tensor.matmul(py[:, c00:QT_W], v_blk(b, kb0),
                                     at[:, c00:QT_W],
                                     start=(kb0 == 0), stop=False,
                                     skip_group_check=True)
                    nc.tensor.matmul(py[:, c01:QT_W], v_blk(b, kb1),
                                     at[:, QT_W + c01:2 * QT_W],
                                     start=False, stop=(kb1 == kb_max - 1),
                                     skip_group_check=True)
                pend.append(y_mms)
                if len(pend) > 2:
                    pend.pop(0)()           # y-matmuls lag two pairs behind exp
                # lvl-0 pair add for the denominator tree
                tree_push(tree_add(at[:, 0:QT_W], at[:, QT_W:2 * QT_W]), 1)
                yield                        # weave point

            while pend:
                pend.pop(0)()
            # drain the tree to a single sum tile
            while len(stack) > 1:
                l1, a = stack.pop()
                _, bt = stack.pop()
                stack.append((l1 + 1, tree_add(a, bt)))
            _, s_fin = stack.pop()

            red = p2_nm.tile([128, QT_W], F32, name="red")
            nc.gpsimd.partition_all_reduce(red[:], s_fin[:], channels=128,
                                           reduce_op=bass_isa.ReduceOp.add)
            rep = p2_nm.tile([128, QT_W], F32, name="rep")
            with nc.allow_low_precision(reason="fast recip"):
                nc.vector.reciprocal_approx_fast(rep[:], red[:])
            yT = p2_y.tile([128, QT_W], BF16, name="yT")
            with nc.allow_low_precision(reason="bf16 y"):
                nc.vector.tensor_tensor(yT[:], py[:], rep[:], op=OP.mult)
            j = 4 * b + qt
            r0 = (TPC // 2) * j + 128 * (h % 2)
            nc.sync.dma_start(a2a_in[hp][r0:r0 + 128, :], yT[:])
            yield

        # ---------------- phase 1 (scoped pools, closed before phase 4)
        with ExitStack() as ctx1:
            p1_w = ctx1.enter_context(tc.tile_pool(name="p1_w", bufs=1))
            p1_x = ctx1.enter_context(tc.tile_pool(name="p1_x", bufs=2))
            p1_x1 = ctx1.enter_context(tc.tile_pool(name="p1_x1", bufs=2))
            p1_cs = ctx1.enter_context(tc.tile_pool(name="p1_cs", bufs=2))
            p1_a = ctx1.enter_context(tc.tile_pool(name="p1_a", bufs=2))
            p1_b = ctx1.enter_context(tc.tile_pool(name="p1_b", bufs=2))
            p1_vs = ctx1.enter_context(tc.tile_pool(name="p1_vs", bufs=2))
            p1_ps = ctx1.enter_context(
                tc.tile_pool(name="p1_ps", bufs=2, space="PSUM"))

            w_sb = [None] * NPAIR

            def emit_w_dma(i):
                wt = p1_w.tile([128, 2 * NROW], BF16, name=f"w{i}")
                nc.sync.dma_start(
                    wt[:].rearrange("p (two c) -> p two c", two=2),
                    wqkvT_d[256 * i:256 * (i + 1), :]
                    .rearrange("(two p) c -> p two c", p=128))
                w_sb[i] = wt

            def wv(kb, m):    # lhsT [128, 128] for (contraction kb, out m)
                return w_sb[kb // 2][:, NROW * (kb % 2) + 128 * m:
                                     NROW * (kb % 2) + 128 * (m + 1)]

            x_tiles = {}

            def emit_x_dma(nt, half):
                c0 = NT * nt
                if half == 0:
                    cs = p1_cs.tile([128, 2, NT], BF16, name="cs")
                    nc.sync.dma_start(cs[:], csP_d[:, :, c0:c0 + NT])
                    x_tiles[nt] = ([None] * 8, cs)
                ts = x_tiles[nt][0]
                for i in range(4 * half, 4 * half + 4):
                    pool_i = p1_x if i < 4 else p1_x1
                    xt = pool_i.tile([128, 4, NT], BF16, name=f"x_{i}")
                    nc.sync.dma_start(
                        xt[:],
                        xT_d[512 * i:512 * (i + 1), c0:c0 + NT]
                        .rearrange("(j p) c -> p j c", p=128))
                    ts[i] = xt

            def p1_chunks(nt, m_order=None):
                """Closures: per m, 4 octet-matmul groups + a rope/v tail."""
                c0 = NT * nt
                ts, cs = x_tiles[nt]
                state = {}
                chunks = []

                def xv(kb):
                    return ts[kb // 4][:, kb % 4, :]

                for m in (m_order or range(NM)):
                    def mk_oct(m, q):
                        def mm_oct():
                            if q == 0:
                                state["pa"] = p1_ps.tile([128, NT], F32,
                                                         name="pa")
                            pa = state["pa"]
                            for kb in range(8 * q, 8 * q + 8):
                                nc.tensor.matmul(pa[:], wv(kb, m), xv(kb),
                                                 start=(kb == 0),
                                                 stop=(kb == KB_D - 1))
                        return mm_oct
                    for q in range(4):
                        chunks.append((1.7, mk_oct(m, q)))

                    if m < NM - 1:
                        def mk_copy(m):
                            def rope_copy():
                                pa = state["pa"]
                                a_t = p1_a.tile([128, NT], BF16, name="a_t")
                                with nc.allow_low_precision(reason="rope"):
                                    nc.scalar.copy(a_t[:], pa[:])
                                state[f"a{m}"] = a_t
                            return rope_copy

                        def mk_pe(m):
                            def rope_pe():
                                a_t = state[f"a{m}"]
                                with nc.allow_low_precision(reason="rope"):
                                    b_t = p1_b.tile([128, NT], BF16,
                                                    name="b_t")
                                    av = a_t[:].rearrange(
                                        "(x two) c -> x two c", two=2)
                                    bv = b_t[:].rearrange(
                                        "(x two) c -> x two c", two=2)
                                    nc.sync.dma_start(bv[:, 0, :],
                                                      av[:, 1, :])
                                    nc.sync.dma_start(bv[:, 1, :],
                                                      av[:, 0, :])
                                    tc_t = p1_a.tile([128, NT], BF16,
                                                     name="tc_t")
                                    nc.vector.tensor_tensor(
                                        tc_t[:], a_t[:], cs[:, 0, :],
                                        op=OP.mult)
                                    ts_t = p1_b.tile([128, NT], BF16,
                                                     name="ts_t")
                                    nc.vector.tensor_tensor(
                                        ts_t[:], b_t[:], cs[:, 1, :],
                                        op=OP.mult)
                                    nc.vector.tensor_tensor(
                                        qkv_sb[m][:, c0:c0 + NT],
                                        tc_t[:], ts_t[:], op=OP.add)
                            return rope_pe
                        chunks.append((0.05, mk_copy(m)))
                        state[f"defer{m}"] = True
                        chunks.append(("defer", (0.2, mk_pe(m))))
                    else:
                        def v_tail():
                            pa = state["pa"]
                            vs = p1_vs.tile([128, NT], BF16, name="vs")
                            with nc.allow_low_precision(reason="bf16 v"):
                                nc.scalar.copy(vs[:], pa[:])
                            nc.scalar.dma_start_transpose(v_sb[nt][:], vs[:])
                        chunks.append((0.05, v_tail))
                # move each deferred rope-PE part past the next m's first
                # octet so the PE never waits on the ACT copy
                final = []
                pending = None
                oct_since = 99
                for c in chunks:
                    if c[0] == "defer":
                        pending = c[1]
                        oct_since = 0
                        continue
                    final.append(c)
                    if pending is not None:
                        oct_since += 1
                        if oct_since >= 3:
                            final.append(pending)
                            pending = None
                if pending is not None:
                    final.append(pending)
                return final

            # unit U(h,b,qt) is eligible after nt block (8b + 2qt + 1) for
            # causal, or after the whole batch otherwise.
            units_after = {nt: [] for nt in range(NNT)}
            for b in range(B):
                for qt in range(N_QT):
                    nt_req = (4 * b + qt) if mode == "causal" \
                        else (4 * b + 3)
                    for h in range(QH):
                        units_after[nt_req].append((h, b, qt))

            def p1_chunks0():
                """nt0 in quad-major order: all 6 m-accumulations advance as
                each (w-pair, x-quad) lands; borrows psc-pool PSUM tiles."""
                ts, cs = x_tiles[0]
                acc = {}
                chunks = []

                def mk_quad(j):
                    def quad():
                        if j == 0:
                            acc[0] = p1_ps.tile([128, NT], F32, name="pa")
                            acc[1] = p1_ps.tile([128, NT], F32, name="pa")
                            w0 = p2_psc.tile([128, 2 * QT_W], F32, name="psc")
                            w1 = p2_psc.tile([128, 2 * QT_W], F32, name="psc")
                            acc[2] = w1[:, NT:2 * NT]
                            acc[3] = w0[:, 0:NT]
                            acc[4] = w0[:, NT:2 * NT]
                            acc[5] = w1[:, 0:NT]
                        for m in range(NM):
                            pa = acc[m] if m >= 3 else acc[m][:]
                            for kb in range(4 * j, 4 * j + 4):
                                nc.tensor.matmul(pa, wv(kb, m),
                                                 ts[j][:, kb % 4, :],
                                                 start=(kb == 0),
                                                 stop=(kb == KB_D - 1),
                                                 skip_group_check=True)
                    return quad
                for j in range(8):
                    chunks.append((5.1, mk_quad(j)))

                ropes = []
                for m in range(5):
                    def mk_copy0(m):
                        def rope_copy():
                            a_t = p1_a.tile([128, NT], BF16, name="a_t")
                            with nc.allow_low_precision(reason="rope"):
                                nc.scalar.copy(
                                    a_t[:], acc[m] if m >= 3 else acc[m][:])
                            acc[f"a{m}"] = a_t
                        return rope_copy

                    def mk_pe0(m):
                        def rope_pe():
                            a_t = acc[f"a{m}"]
                            with nc.allow_low_precision(reason="rope"):
                                b_t = p1_b.tile([128, NT], BF16, name="b_t")
                                av = a_t[:].rearrange(
                                    "(x two) c -> x two c", two=2)
                                bv = b_t[:].rearrange(
                                    "(x two) c -> x two c", two=2)
                                nc.sync.dma_start(bv[:, 0, :], av[:, 1, :])
                                nc.sync.dma_start(bv[:, 1, :], av[:, 0, :])
                                tc_t = p1_a.tile([128, NT], BF16, name="tc_t")
                                nc.vector.tensor_tensor(
                                    tc_t[:], a_t[:], cs[:, 0, :], op=OP.mult)
                                ts_t = p1_b.tile([128, NT], BF16, name="ts_t")
                                nc.vector.tensor_tensor(
                                    ts_t[:], b_t[:], cs[:, 1, :], op=OP.mult)
                                nc.vector.tensor_tensor(
                                    qkv_sb[m][:, 0:NT],
                                    tc_t[:], ts_t[:], op=OP.add)
                        return rope_pe
                    ropes.append((mk_copy0(m), mk_pe0(m)))

                def v_tail0():
                    vs = p1_vs.tile([128, NT], BF16, name="vs")
                    with nc.allow_low_precision(reason="bf16 v"):
                        nc.scalar.copy(vs[:], acc[5])
                    nc.scalar.dma_start_transpose(v_sb[0][:], vs[:])

                # rope m2 first: frees the pb-pool accumulator slot that the
                # first rope_pe needs for its pswap output
                order = [2, 0, 1, 3, 4]
                chunks.append((0.05, ropes[order[0]][0]))
                chunks.append((0.05, ropes[order[1]][0]))
                for i in range(5):
                    chunks.append((0.2, ropes[order[i]][1]))
                    if i + 2 < 5:
                        chunks.append((0.05, ropes[order[i + 2]][0]))
                chunks.append((0.05, v_tail0))
                return chunks

            # progressive startup: w pairs and x quads interleaved in
            # kb-consumption order (kb quad j needs w pairs 2j,2j+1, x quad j)
            c0 = 0
            cs0 = p1_cs.tile([128, 2, NT], BF16, name="cs")
            x_tiles[0] = ([None] * 8, cs0)
            for j in range(8):
                emit_w_dma(2 * j)
                emit_w_dma(2 * j + 1)
                xt = p1_x.tile([128, 4, NT], BF16, name=f"x_{j}") if j < 4 \
                    else p1_x1.tile([128, 4, NT], BF16, name=f"x_{j}")
                nc.sync.dma_start(
                    xt[:],
                    xT_d[512 * j:512 * (j + 1), 0:NT]
                    .rearrange("(jj p) c -> p jj c", p=128))
                x_tiles[0][0][j] = xt
                if j == 0:
                    nc.sync.dma_start(cs0[:], csP_d[:, :, 0:NT])
                    emit_const_dmas()
            filler = []

            pending_units = []
            tail_units = []
            extended = set()

            def extend_chunks(nt):
                if nt in extended or nt >= NNT:
                    return
                extended.add(nt)
                mo = [4, 5, 0, 1, 2, 3] if nt == NNT - 1 else None
                for (w, c) in (p1_chunks0() if nt == 0
                               else p1_chunks(nt, mo)):
                    filler.append((nt, w, c))

            def fill1(cur_nt, need=0.8):
                got = 0.0
                while got < need:
                    if not filler:
                        extend_chunks(cur_nt + 1)
                        if not filler:
                            return
                    nt_, w, c = filler.pop(0)
                    c()
                    got += w

            for nt in range(NNT):
                extend_chunks(nt)
                if nt + 1 < NNT:
                    emit_x_dma(nt + 1, 0)
                    emit_x_dma(nt + 1, 1)
                while pending_units:
                    gen = pending_units.pop(0)
                    for _ in gen:
                        fill1(nt)
                if nt == NNT - 1:
                    # flush only kT/v/hp0-head chunks (first 4 m-groups);
                    # the hp1-head chunks stay as filler for the hp0 units
                    n_keep = 2 * 6   # m2,m3: 4 octets + copy + rope-pe each
                    while len(filler) > n_keep and filler[0][0] == nt:
                        filler.pop(0)[2]()
                else:
                    while filler and filler[0][0] == nt:
                        filler.pop(0)[2]()
                for (h, b, qt) in units_after[nt]:
                    held = (b == 1 and h >= 2 and
                            (qt == N_QT - 1 or
                             (qt == N_QT - 2 and mode == "causal")))
                    if not held:
                        pending_units.append(attn_unit(h, b, qt))
                    else:
                        tail_units.append((h, b, qt))
            # hp0 units of the final round, woven with the hp1-head chunks
            for gen in pending_units:
                for _ in gen:
                    fill1(NNT)
            while filler:
                filler.pop(0)[2]()
            pending_units = []

        # ---------------- A2A #1 (hp0) + two-pass phase 4
        # pass 1 (hp0 head-blocks, needs only A2A#1) runs as PE filler under
        # the ACT-bound hp1 attention tail, spilling f32 partials to out_d;
        # pass 2 (after A2A#2) adds the hp1 blocks and the reloaded partials.
        def emit_a2a(hp):
            if sim:
                for j in range(CORES):
                    nc.sync.dma_start(
                        a2a_out[hp][(TPC // 2) * j:(TPC // 2) * (j + 1), :],
                        a2a_in[hp][(TPC // 2) * j:(TPC // 2) * (j + 1), :])
            else:
                nc.gpsimd.collective_compute(
                    "AllToAll", mybir.AluOpType.bypass,
                    replica_groups=[list(range(CORES))],
                    ins=[a2a_in[hp][:]], outs=[a2a_out[hp][:]],
                )

        emit_a2a(0)

        p4_y = glob.enter_context(tc.tile_pool(name="p4_y", bufs=1))
        p4_w = glob.enter_context(tc.tile_pool(name="p4_w", bufs=2))
        p4_o = glob.enter_context(tc.tile_pool(name="p4_o", bufs=2))
        p4_i = glob.enter_context(tc.tile_pool(name="p4_i", bufs=2))
        p4_po = glob.enter_context(
            tc.tile_pool(name="p4_po", bufs=2, space="PSUM"))

        y_all = [None, None]
        y_all[0] = p4_y.tile([128, 16, TPC], BF16, name="y_all0")
        nc.sync.dma_start(
            y_all[0][:], a2a_out[0][:].rearrange("(k p) t -> p k t", p=128))

        def y_v(g):           # lhsT rows for global head-block g
            i, hl = g // 4, g % 4
            return y_all[hl // 2][:, 2 * i + (hl % 2), :]

        gs_h0 = [g for g in range(KB_D) if (g % 4) < 2]
        gs_h1 = [g for g in range(KB_D) if (g % 4) >= 2]
        n_do = D // WO_NT

        wo_tiles = {}

        def emit_wo_dma(do, half):
            wt = p4_w.tile([128, 16, WO_NT], BF16, name="wo")
            for sub in range(2):
                nc.sync.dma_start(
                    wt[:].rearrange("p (i two) c -> p i two c", two=2)
                    [:, :, sub, :],
                    woT_d[:, WO_NT * do:WO_NT * (do + 1)]
                    .rearrange("(i hl p) c -> p i hl c", hl=4, p=128)
                    [:, :, 2 * half + sub, :])
            wo_tiles[(do, half)] = wt

        def out_slice(do, d=None):
            return (d if d is not None else out_d) \
                [:, WO_NT * do:WO_NT * (do + 1)] \
                .rearrange("(tb p) c -> p tb c", p=128)

        def half_tiles(do, half, add_partial, part=None):
            wt = wo_tiles[(do, half)]
            gs = gs_h0 if half == 0 else gs_h1
            ob = p4_o.tile([128, 4, WO_NT], BF16, name="ob")
            for tb in range(4):
                po = p4_po.tile([128, WO_NT], F32, name="po")
                for n, g in enumerate(gs):
                    nc.tensor.matmul(po[:],
                                     y_v(g)[:, 128 * tb:128 * (tb + 1)],
                                     wt[:, n, :], start=(n == 0),
                                     stop=(n == 15), skip_group_check=True)
                with nc.allow_low_precision(reason="bf16 partial"):
                    if add_partial:
                        nc.vector.tensor_tensor(ob[:, tb, :], po[:],
                                                part[:, tb, :], op=OP.add)
                        nc.sync.dma_start(out_slice(do)[:, tb, :],
                                          ob[:, tb, :])
                    else:
                        nc.vector.tensor_copy(ob[:, tb, :], po[:])
            if not add_partial:
                nc.sync.dma_start(out_slice(do, part_d), ob[:])

        # pass-1 closures at do granularity (PE filler for the tail units)
        emit_wo_dma(0, 0)
        pass1 = []
        for do in range(n_do):
            def mk(do=do):
                def go():
                    if do + 1 < n_do:
                        emit_wo_dma(do + 1, 0)
                    half_tiles(do, 0, False)
                return go
            pass1.append(mk())

        p1_idx = 0
        for (h, b, qt) in tail_units:
            for _ in attn_unit(h, b, qt):
                if p1_idx < len(pass1) and p1_idx * 2 < len(pass1):
                    pass1[p1_idx]()
                    p1_idx += 1
        while p1_idx < len(pass1):
            pass1[p1_idx]()
            p1_idx += 1

        emit_a2a(1)
        y_all[1] = p4_y.tile([128, 16, TPC], BF16, name="y_all1")
        nc.sync.dma_start(
            y_all[1][:], a2a_out[1][:].rearrange("(k p) t -> p k t", p=128))

        # ---------------- pass 2: hp1 blocks + partial reload
        emit_wo_dma(0, 1)

        def load_part(do):
            part = p4_i.tile([128, 4, WO_NT], BF16, name="part")
            nc.sync.dma_start(part[:], out_slice(do, part_d))
            return part

        part = load_part(0)
        for do in range(n_do):
            nxt = load_part(do + 1) if do + 1 < n_do else None
            if do + 1 < n_do:
                emit_wo_dma(do + 1, 1)
            half_tiles(do, 1, True, part)
            part = nxt

    nc.compile()
    return nc


def _prepare(x, freqs_cis, mask, wqkv_w, wo_w):
    """Host-side prep: mode detection, stability constant, input maps."""
    import ml_dtypes
    bf16 = ml_dtypes.bfloat16
    x = np.asarray(x, dtype=np.float32)
    freqs_cis = np.asarray(freqs_cis, dtype=np.float32)
    mask = np.asarray(mask)
    wqkv_w = np.asarray(wqkv_w, dtype=np.float32)
    wo_w = np.asarray(wo_w, dtype=np.float32)

    m2 = mask.reshape(mask.shape[-2], mask.shape[-1])
    if np.array_equal(m2, np.tril(np.ones((S, S), dtype=bool))):
        mode = "causal"
    elif m2.all():
        mode = "full"
    else:
        mode = "generic"

    x2 = x.reshape(TOK, D)
    xT = np.ascontiguousarray(x2.T.astype(bf16))
    woT = np.ascontiguousarray(wo_w.T.astype(bf16))

    cos = freqs_cis[:, :, 0].T          # [64, S]
    sin = freqs_cis[:, :, 1].T
    cosP = np.repeat(cos, 2, axis=0)    # [128, S]
    sinP = np.repeat(sin, 2, axis=0)
    # swap is sign-free (plain pair swap on the DMA); fold the rotation
    # signs into sin: even hd rows use -sin, odd rows +sin
    sgn = np.tile(np.array([-1.0, 1.0], np.float32), 64)[:, None]
    csP = np.stack([np.tile(cosP, (1, B)), np.tile(sinP * sgn, (1, B))],
                   axis=1).astype(bf16)  # [128, 2, TOK]
    csP = np.ascontiguousarray(csP)

    # softmax stability probe: rope'd scores for head 0, batch 0, 128 q rows
    wq0 = wqkv_w[:HD]
    wk0 = wqkv_w[NH * HD:NH * HD + HD]
    qs = x2[:128] @ wq0.T
    ks = x2[:S] @ wk0.T

    def rope_np(t, fc):
        ts = t.reshape(t.shape[0], HD // 2, 2)
        c, s_ = fc[:t.shape[0], :, 0], fc[:t.shape[0], :, 1]
        out = np.empty_like(ts)
        out[:, :, 0] = ts[:, :, 0] * c - ts[:, :, 1] * s_
        out[:, :, 1] = ts[:, :, 1] * c + ts[:, :, 0] * s_
        return out.reshape(t.shape)

    qs = rope_np(qs, freqs_cis)
    ks = rope_np(ks, freqs_cis)
    smax = float(np.max(np.abs(qs @ ks.T)) * SCALE)
    c_sub = 0.0 if smax < 25.0 else smax + 5.0

    in_maps = []
    for c in range(CORES):
        wq_c = wqkv_w[QH * HD * c:QH * HD * (c + 1)]
        wk_c = wqkv_w[NH * HD + HD * c:NH * HD + HD * (c + 1)]
        wv_c = wqkv_w[(NH + NL) * HD + HD * c:(NH + NL) * HD + HD * (c + 1)]
        wqkvT_c = np.ascontiguousarray(
            np.vstack([wq_c, wk_c, wv_c]).T.astype(bf16))
        m = {"xT": xT, "wqkvT": wqkvT_c, "woT": woT, "csP": csP}
        if mode == "generic":
            m["biasT"] = np.ascontiguousarray(np.where(
                m2.T, np.float32(0), np.float32(-1e30)).astype(bf16))
        in_maps.append(m)
    return mode, c_sub, in_maps


def _get_nc(mode, c_sub):
    key = (mode, round(float(c_sub), 3))
    if key not in _CACHE:
        _CACHE[key] = _build_nc(mode, c_sub)
    return _CACHE[key]


def kernel(x, freqs_cis, mask, wqkv_w, wo_w):
    from concourse import bass_utils
    mode, c_sub, in_maps = _prepare(x, freqs_cis, mask, wqkv_w, wo_w)
    nc = _get_nc(mode, c_sub)
    res = bass_utils.run_bass_kernel_spmd(nc, in_maps, core_ids=list(range(CORES)))
    out = np.concatenate([np.asarray(res.results[c]["out"], dtype=np.float32)
                          for c in range(CORES)], axis=0)
    return out.reshape(B, S, D)
